# revision 26
# baseline (speedup 1.0000x reference)
"""Fused multi-head causal attention block (qkv proj + attention + out proj)
for Trainium2, data-parallel over batch across 8 NeuronCores.

Contract: kernel(**inputs) takes the FULL inputs
    x [8,1024,1024] f32, Wqkv [1024,3072], bqkv [3072], Wproj [1024,1024],
    bproj [1024]
and returns (a, present) exactly like the reference:
    a [8,1024,1024] f32, present [2,8,16,1024,64] f32.

Per-core program (SPMD, one batch element per core), all matmuls float32r:
  stage A: k^T (transposed [hd, s]) and v (natural [s, hd], padded with a
           ones column per head) projections; x^T is pre-transposed on host.
  attn:    processed per head PAIR with the q-projection FUSED in (keeps the
           PE stream dense with full-array matmuls so the HAM clock-gate
           stays at full rate): project the pair's q rows, then scores
           w^T[t,s] = (k^T).T @ q^T — even head on PE rows 0..63, odd head
           on rows 64..127, emitted adjacently inside a tile_critical so
           they run concurrently on disjoint row groups; causal blocks are
           skipped and partial blocks narrowed to the valid column range;
           exp applies the 1/sqrt(D) scale; context via the [v|1] fused
           stationary also yields the softmax denominator in row 64.
           Denominators collect into one tile (ACT row copy + tiny DMA);
           one batched reciprocal; a K=16 selector matmul broadcasts the
           recips for a pair; one DVE multiply normalizes in place.
  proj:    out[s,m] = sum over 128-row hd chunks  aT[c].T @ Wproj[c] + bias.

Biases are applied exactly via K=1 rank-1 update matmuls (they are zeros in
this problem but handled generally).
"""

import numpy as np

import concourse.bass as bass
import concourse.mybir as mybir
import concourse.tile as tile
from concourse import bacc

B, S, NX, H, D = 8, 1024, 1024, 16, 64
P = 128
E = D + 1  # v columns per head incl. ones column
F32 = mybir.dt.float32
FR = mybir.dt.float32r
AF = mybir.ActivationFunctionType
OP = mybir.AluOpType


def _fr(ap):
    return ap.bitcast(FR)


def build_nc():
    nc = bacc.Bacc("TRN2", target_bir_lowering=False)

    xT_d = nc.dram_tensor("xT", [NX, S], F32, kind="ExternalInput")
    wqkv_d = nc.dram_tensor("Wqkv", [NX, 3 * NX], F32, kind="ExternalInput")
    bqkv_d = nc.dram_tensor("bqkv", [1, 3 * NX], F32, kind="ExternalInput")
    wproj_d = nc.dram_tensor("Wproj", [NX, NX], F32, kind="ExternalInput")
    bproj_d = nc.dram_tensor("bproj", [1, NX], F32, kind="ExternalInput")
    mask_d = nc.dram_tensor("cmask", [P, P], F32, kind="ExternalInput")
    ones_d = nc.dram_tensor("ones", [1, S], F32, kind="ExternalInput")
    vones_d = nc.dram_tensor("vones", [P, H], F32, kind="ExternalInput")
    sel8_d = nc.dram_tensor("sel8", [8, H, P], F32, kind="ExternalInput")
    outa_d = nc.dram_tensor("out_a", [S, NX], F32, kind="ExternalOutput")
    outk_d = nc.dram_tensor("out_kT", [NX, S], F32, kind="ExternalOutput")
    outv_d = nc.dram_tensor("out_v", [S, NX], F32, kind="ExternalOutput")

    with tile.TileContext(nc) as tc:
        with (
            tc.tile_pool(name="const", bufs=1) as constp,
            tc.tile_pool(name="xp", bufs=1) as xp,
            tc.tile_pool(name="kv", bufs=1, side="right") as kvp,
            tc.tile_pool(name="atp", bufs=1) as atp,
        ):
            # all-ones rows (memset cannot produce fp32r-rounded data, so
            # matmul-consumed constants are DMA'd from DRAM inputs)
            ones_sb = constp.tile([1, S], F32, name="ones_sb")
            nc.sync.dma_start(out=_fr(ones_sb), in_=_fr(ones_d[:, :]))
            # q-part bias row
            bq_q = constp.tile([1, NX], F32, name="bq_q")
            nc.sync.dma_start(out=_fr(bq_q), in_=_fr(bqkv_d[:, 0:NX]))
            # triangular causal mask for diagonal score blocks
            mask_sb = constp.tile([P, P], F32, name="mask_sb")
            nc.sync.dma_start(out=mask_sb, in_=mask_d[:, :])
            # normalize selectors: sel8[c8][r, p] = 1 iff r == 2*c8 + (p>=64)
            sel8_sb = [
                constp.tile([H, P], F32, name=f"sel8_{c}", tag=f"sel8_{c}")
                for c in range(8)
            ]
            for c in range(8):
                nc.sync.dma_start(out=_fr(sel8_sb[c]), in_=_fr(sel8_d[c]))

            xT = []
            for n in range(8):
                t = xp.tile([P, S], F32, name=f"xT{n}", tag=f"xT{n}")
                nc.sync.dma_start(out=_fr(t), in_=_fr(xT_d[n * P : (n + 1) * P, :]))
                xT.append(t)

            kT = [kvp.tile([P, S], F32, name=f"kT{i}", tag=f"kT{i}") for i in range(8)]
            vpad = [
                kvp.tile([P, H * E], F32, name=f"vp{i}", tag=f"vp{i}")
                for i in range(8)
            ]
            # head pair c: head 2c on partitions 0..63, head 2c+1 on 64..127
            aT = [atp.tile([P, S], F32, name=f"aT{c}", tag=f"aT{c}") for c in range(8)]
            den_all = atp.tile([H, S], F32, name="den_all")
            rall = atp.tile([H, S], F32, name="rall")

            # ------------- stage A: k^T and v projections -------------
            with (
                tc.tile_pool(name="wgp", bufs=10) as wgp,
                tc.tile_pool(name="psA", bufs=5, space="PSUM") as psA,
            ):
                for t in range(8):
                    v3 = vpad[t].rearrange("p (h e) -> p h e", e=E)
                    nc.sync.dma_start(
                        out=_fr(v3[:, :, D : D + 1]), in_=_fr(vones_d[:, :, None])
                    )
                for mg in range(2, 6):
                    wg = []
                    for n in range(8):
                        w = wgp.tile([P, 512], F32, name=f"wg_{mg}_{n}", tag="wg")
                        nc.sync.dma_start(
                            out=_fr(w),
                            in_=_fr(
                                wqkv_d[n * P : (n + 1) * P, mg * 512 : (mg + 1) * 512]
                            ),
                        )
                        wg.append(w)
                    bqg = wgp.tile([1, 512], F32, name=f"bqg_{mg}", tag="bqg", bufs=3)
                    nc.sync.dma_start(
                        out=_fr(bqg), in_=_fr(bqkv_d[:, mg * 512 : (mg + 1) * 512])
                    )
                    if mg < 4:
                        # k^T, transposed layout [m, s]
                        for ml in range(4):
                            ki = (mg - 2) * 4 + ml
                            for j in range(2):
                                ps = psA.tile(
                                    [P, 512], F32, tag="pa", name=f"pa_{mg}_{ml}_{j}"
                                )
                                for n in range(8):
                                    nc.tensor.matmul(
                                        ps,
                                        _fr(wg[n][:, ml * P : (ml + 1) * P]),
                                        _fr(xT[n][:, j * 512 : (j + 1) * 512]),
                                        start=(n == 0),
                                        stop=False,
                                    )
                                nc.tensor.matmul(
                                    ps,
                                    _fr(bqg[:, ml * P : (ml + 1) * P]),
                                    _fr(ones_sb[:, j * 512 : (j + 1) * 512]),
                                    start=False,
                                    stop=True,
                                )
                                nc.vector.tensor_copy(
                                    _fr(kT[ki][:, j * 512 : (j + 1) * 512]), ps
                                )
                            nc.sync.dma_start(
                                out=outk_d[ki * P : (ki + 1) * P, :], in_=kT[ki]
                            )
                    else:
                        # v, natural layout [t, hd] with ones column per head
                        mvg = mg - 4
                        h0 = mvg * 8
                        for t in range(8):
                            ps = psA.tile([P, 512], F32, tag="pa", name=f"pv_{mg}_{t}")
                            for n in range(8):
                                nc.tensor.matmul(
                                    ps,
                                    _fr(xT[n][:, t * P : (t + 1) * P]),
                                    _fr(wg[n]),
                                    start=(n == 0),
                                    stop=False,
                                )
                            nc.tensor.matmul(
                                ps,
                                _fr(ones_sb[:, t * P : (t + 1) * P]),
                                _fr(bqg),
                                start=False,
                                stop=True,
                            )
                            v3 = vpad[t].rearrange("p (h e) -> p h e", e=E)
                            nc.vector.tensor_copy(
                                _fr(v3[:, h0 : h0 + 8, 0:D]),
                                ps.rearrange("p (h d) -> p h d", d=D),
                            )
                            if mvg == 1:
                                nc.sync.dma_start(
                                    out=outv_d[t * P : (t + 1) * P, :].rearrange(
                                        "p (h d) -> p h d", d=D
                                    ),
                                    in_=v3[:, :, 0:D],
                                )

            # ------------- attention with fused q-projection -------------
            with (
                tc.tile_pool(name="wqp", bufs=12) as wqp,
                tc.tile_pool(name="qsp", bufs=2) as qsp,
                tc.tile_pool(name="rcp", bufs=2) as rcp,
                tc.tile_pool(name="etp", bufs=12) as etp,
                tc.tile_pool(name="psT", bufs=2, space="PSUM") as psT,
            ):
                for c8 in range(8):
                    # project this pair's q rows: q^T[pair, s] (+bias), into qs
                    wq = []
                    for n in range(8):
                        w = wqp.tile([P, P], F32, name=f"wq_{c8}_{n}", tag="wq")
                        nc.sync.dma_start(
                            out=_fr(w),
                            in_=_fr(
                                wqkv_d[n * P : (n + 1) * P, c8 * P : (c8 + 1) * P]
                            ),
                        )
                        wq.append(w)
                    qs = qsp.tile([P, S], F32, name=f"qs_{c8}", tag="qs")
                    for j in range(2):
                        psq = psT.tile(
                            [P, 512], F32, tag="ps_q", bufs=2, name=f"psq_{c8}_{j}"
                        )
                        for n in range(8):
                            nc.tensor.matmul(
                                psq,
                                _fr(wq[n]),
                                _fr(xT[n][:, j * 512 : (j + 1) * 512]),
                                start=(n == 0),
                                stop=False,
                            )
                        nc.tensor.matmul(
                            psq,
                            _fr(bq_q[:, c8 * P : (c8 + 1) * P]),
                            _fr(ones_sb[:, j * 512 : (j + 1) * 512]),
                            start=False,
                            stop=True,
                        )
                        nc.vector.tensor_copy(
                            _fr(qs[:, j * 512 : (j + 1) * 512]), psq
                        )
                    for j in range(2):
                        ntc = 4 * j + 4  # causal: t-chunks 0..4j+3
                        es = {0: [], 1: []}
                        offs = []
                        for c in range(ntc):
                            # columns left of the diagonal are fully masked;
                            # compute only the causal-valid range [lo, 512)
                            off = c * P - j * 512
                            lo = max(0, off)
                            offs.append(lo)
                            pss = {}
                            # even head rows 0..63, odd rows 64..127: adjacent
                            # matmuls on disjoint PE row groups run concurrently
                            with tc.tile_critical():
                                for r in range(2):
                                    qr = r * D
                                    pss[r] = psT.tile(
                                        [P, 512],
                                        F32,
                                        tag=f"ps_s{r}",
                                        bufs=2,
                                        name=f"pss_{c8}_{j}_{c}_{r}",
                                    )
                                    nc.tensor.matmul(
                                        pss[r][:, lo:512],
                                        _fr(kT[c8][qr : qr + D, c * P : (c + 1) * P]),
                                        _fr(
                                            qs[
                                                qr : qr + D,
                                                j * 512 + lo : (j + 1) * 512,
                                            ]
                                        ),
                                        start=True,
                                        stop=True,
                                    )
                            for r in range(2):
                                if off >= 0:  # diagonal block: tri mask add
                                    nc.vector.tensor_tensor(
                                        pss[r][:, off : off + P],
                                        pss[r][:, off : off + P],
                                        mask_sb,
                                        OP.add,
                                    )
                                e = etp.tile(
                                    [P, 512], F32, tag="et", name=f"e_{c8}_{j}_{c}_{r}"
                                )
                                # scale=0.125 applies the 1/sqrt(D) factor
                                nc.scalar.activation(
                                    _fr(e[:, lo:512]),
                                    pss[r][:, lo:512],
                                    AF.Exp,
                                    scale=0.125,
                                )
                                es[r].append(e)
                        # context rows 0..63 + denominator row 64, via the
                        # [v|1] fused stationary (matmul dst at partition 0)
                        psc = {}
                        for r in range(2):
                            h = 2 * c8 + r
                            psc[r] = psT.tile(
                                [P, 512], F32, tag=f"ps_c{r}", bufs=1,
                                name=f"psc_{c8}_{j}_{r}",
                            )
                            for c in range(ntc):
                                lo = offs[c]
                                nc.tensor.matmul(
                                    psc[r][0:E, lo:512],
                                    _fr(vpad[c][:, h * E : h * E + E]),
                                    _fr(es[r][c][:, lo:512]),
                                    start=(c == 0),
                                    stop=(c == ntc - 1),
                                )
                        for r in range(2):
                            h = 2 * c8 + r
                            # stash the denominator row (ACT: PSUM row 64 ->
                            # SBUF row 64, tiny DMA to partition h of den_all)
                            rec = rcp.tile(
                                [P, 512], F32, tag="rec", name=f"rec_{h}_{j}"
                            )
                            nc.scalar.activation(
                                rec[D : D + 1, :], psc[r][D : D + 1, :], AF.Copy
                            )
                            nc.sync.dma_start(
                                out=den_all[h : h + 1, j * 512 : (j + 1) * 512],
                                in_=rec[D : D + 1, :],
                            )
                            # evict the unnormalized context rows
                            if r == 0:
                                nc.vector.tensor_copy(
                                    _fr(aT[c8][0:D, j * 512 : (j + 1) * 512]),
                                    psc[r][0:D, :],
                                )
                            else:
                                # odd head rows go to partitions 64..127 of the
                                # pair tile; DVE is lane-bound: stage + DMA
                                tmp = rcp.tile(
                                    [D, 512], F32, tag="tmp", name=f"tmp_{h}_{j}"
                                )
                                nc.vector.tensor_copy(_fr(tmp), psc[r][0:D, :])
                                nc.sync.dma_start(
                                    out=_fr(
                                        aT[c8][D : D + D, j * 512 : (j + 1) * 512]
                                    ),
                                    in_=_fr(tmp),
                                )
                # one batched reciprocal over all (head, s) denominators
                with nc.allow_low_precision(
                    reason="fp32r rounding of softmax reciprocal rows"
                ):
                    nc.vector.reciprocal(_fr(rall), den_all)

            # normalize: K=16 selector matmul broadcasts each pair's two
            # recip rows across its 128 partitions, then one DVE multiply
            with tc.tile_pool(name="psN", bufs=2, space="PSUM") as psN:
                for c8 in range(8):
                    for j in range(2):
                        psb = psN.tile(
                            [P, 512], F32, tag="ps_b", bufs=2, name=f"psb_{c8}_{j}"
                        )
                        nc.tensor.matmul(
                            psb,
                            _fr(sel8_sb[c8]),
                            _fr(rall[:, j * 512 : (j + 1) * 512]),
                            start=True,
                            stop=True,
                        )
                        nc.vector.tensor_tensor(
                            _fr(aT[c8][:, j * 512 : (j + 1) * 512]),
                            aT[c8][:, j * 512 : (j + 1) * 512],
                            psb,
                            OP.mult,
                        )

            # ---------------- output projection ----------------
            with (
                tc.tile_pool(name="wpp", bufs=10) as wpp,
                tc.tile_pool(name="evp", bufs=4) as evp,
                tc.tile_pool(name="psP", bufs=4, space="PSUM") as psP,
            ):
                for mg in range(2):
                    wp = []
                    for n in range(8):
                        w = wpp.tile([P, 512], F32, name=f"wp_{mg}_{n}", tag="wp")
                        nc.sync.dma_start(
                            out=_fr(w),
                            in_=_fr(
                                wproj_d[n * P : (n + 1) * P, mg * 512 : (mg + 1) * 512]
                            ),
                        )
                        wp.append(w)
                    bpg = wpp.tile([1, 512], F32, name=f"bpg_{mg}", tag="bpg", bufs=2)
                    nc.sync.dma_start(
                        out=_fr(bpg), in_=_fr(bproj_d[:, mg * 512 : (mg + 1) * 512])
                    )
                    for i in range(8):
                        ps = psP.tile([P, 512], F32, tag="pp", name=f"pp_{mg}_{i}")
                        for n in range(8):
                            nc.tensor.matmul(
                                ps,
                                _fr(aT[n][:, i * P : (i + 1) * P]),
                                _fr(wp[n]),
                                start=(n == 0),
                                stop=False,
                            )
                        nc.tensor.matmul(
                            ps,
                            _fr(ones_sb[:, i * P : (i + 1) * P]),
                            _fr(bpg),
                            start=False,
                            stop=True,
                        )
                        ev = evp.tile([P, 512], F32, tag="ev", name=f"ev_{mg}_{i}")
                        nc.vector.tensor_copy(ev, ps)
                        nc.sync.dma_start(
                            out=outa_d[i * P : (i + 1) * P, mg * 512 : (mg + 1) * 512],
                            in_=ev,
                        )

    nc.compile()
    return nc


_NC_CACHE = None


def get_nc():
    global _NC_CACHE
    if _NC_CACHE is None:
        _NC_CACHE = build_nc()
    return _NC_CACHE


def make_in_maps(x, Wqkv, bqkv, Wproj, bproj):
    cmask = np.where(
        np.arange(P)[None, :] >= np.arange(P)[:, None], 0.0, -10000.0
    ).astype(np.float32)
    shared = {
        "Wqkv": np.ascontiguousarray(Wqkv, dtype=np.float32),
        "bqkv": np.ascontiguousarray(bqkv, dtype=np.float32).reshape(1, -1),
        "Wproj": np.ascontiguousarray(Wproj, dtype=np.float32),
        "bproj": np.ascontiguousarray(bproj, dtype=np.float32).reshape(1, -1),
        "cmask": cmask,
        "ones": np.ones((1, S), dtype=np.float32),
        "vones": np.ones((P, H), dtype=np.float32),
        "sel8": np.stack(
            [
                np.stack(
                    [
                        (np.full(P, r) == 2 * c + (np.arange(P) >= D)).astype(
                            np.float32
                        )
                        for r in range(H)
                    ]
                )
                for c in range(8)
            ]
        ),
    }
    return [
        {"xT": np.ascontiguousarray(np.asarray(x[b], dtype=np.float32).T), **shared}
        for b in range(B)
    ]


def assemble_outputs(results):
    a = np.stack([r["out_a"] for r in results])  # [B, S, NX]
    k = np.stack(
        [r["out_kT"].reshape(H, D, S).transpose(0, 2, 1) for r in results]
    )  # [B, H, S, D]
    v = np.stack(
        [r["out_v"].reshape(S, H, D).transpose(1, 0, 2) for r in results]
    )  # [B, H, S, D]
    present = np.stack([k, v])  # [2, B, H, S, D]
    return a, present


def kernel(x, Wqkv, bqkv, Wproj, bproj):
    from concourse.bass_utils import run_bass_kernel_spmd

    nc = get_nc()
    in_maps = make_in_maps(x, Wqkv, bqkv, Wproj, bproj)
    res = run_bass_kernel_spmd(nc, in_maps, core_ids=list(range(B)))
    return assemble_outputs(res.results)
